# revision 26
# baseline (speedup 1.0000x reference)
"""CQAttention Trainium2 kernel (v3).

Reference per batch b (C:[D,Lc], Q:[D,Lq], D=128, Lc=2048, Lq=512):
    Ct = C^T, Qt = Q^T
    S  = Ct@w4C + (Qt@w4Q)^T + (Ct*w4mlu)@Qt^T + bias        [Lc, Lq]
    S1 = softmax_q(S + NEG*(1-qmask)),  S2 = softmax_c(S + NEG*(1-cmask))
    A  = S1 @ Qt;  B = S1 @ (S2^T @ Ct)
    out= transpose(concat([Ct, A, Ct*A, Ct*B], -1))           [4D, Lc]

Kernel algebra (single exp pass; one full-S exponential family):
    qterm[q] = q1[q] + qneg[q]   (rank-0/1 matmuls; PSUM-seeded per S tile)
    X[c,q]   = exp(sub2[c,q] + qterm[q] + c0[c] + cneg[c])
    rowsum[c]= sum_q X[c,q]              (free via ActE accum_out on the exp)
    S1t[q,c] = X^T[q,c] / rowsum[c]      (diag(rcp) matmul-transpose: a plain
                                          bf16 matmul, stationary X-block
                                          shared with the R matmuls)
    R        = (X wts vs [Ct|1]) -> rp/(s2+eps)
    A^T      = Qt^T @ S1t ;  B^T = R^T @ S1t
  `bias` cancels in both softmaxes (dropped exactly). The per-c factor
  exp(c0+cneg) cancels in S1's rowsum division; per-q factor exp(qterm)
  cancels in R's s2 division, so one exp family serves both softmaxes.
  qmask semantics are exact (masked q vanish from X; eps-guarded s2 keeps
  R's masked rows at 0, which S1t's zero rows then ignore). For cmask=0
  columns A/CA/CB deviate from the reference (0 instead of unmasked-softmax
  values); the problem's input spec fixes Cmask=ones.
  CA/CB products run on the (otherwise idle) Pool engine from SBUF copies
  of A/B; those copies also feed the output DMA.
"""

import numpy as np
from contextlib import ExitStack, nullcontext

import concourse.bass as bass
import concourse.mybir as mybir
import concourse.tile as tile
from concourse import bacc
from concourse.bass_utils import run_bass_kernel_spmd
from concourse.masks import make_identity

F32 = mybir.dt.float32
F32R = mybir.dt.float32r
I32 = mybir.dt.int32
BF16 = mybir.dt.bfloat16
AF = mybir.ActivationFunctionType
ALU = mybir.AluOpType

B, D, LC, LQ = 16, 128, 2048, 512
NCORES = 8
BL = B // NCORES          # batches per core
NEG = -1e30
NCT = LC // 128           # 16 c-tiles
NQT = LQ // 128           # 4 q-tiles
NCJ = LC // 512           # 4 c-chunks (free-dim)
CTS = 130                 # Ct slot: [Ct(128) | ones(1) | pad(1)]
EPS = 1e-30
HIPRI_OFF = 155


def _build_nc():
    nc = bacc.Bacc("TRN2", target_bir_lowering=False)
    Ci = nc.dram_tensor("C", [BL, D, LC], F32, kind="ExternalInput")
    Qi = nc.dram_tensor("Q", [BL, D, LQ], F32, kind="ExternalInput")
    CMi = nc.dram_tensor("Cmask", [BL, LC], I32, kind="ExternalInput")
    QMi = nc.dram_tensor("Qmask", [BL, LQ], I32, kind="ExternalInput")
    w4C = nc.dram_tensor("w4C", [D, 1], F32, kind="ExternalInput")
    w4Q = nc.dram_tensor("w4Q", [D, 1], F32, kind="ExternalInput")
    w4mlu = nc.dram_tensor("w4mlu", [1, 1, D], F32, kind="ExternalInput")
    biasi = nc.dram_tensor("bias", [1], F32, kind="ExternalInput")
    out = nc.dram_tensor("out", [BL, 4 * D, LC], F32, kind="ExternalOutput")

    with tile.TileContext(nc) as tc, ExitStack() as ctx:
        const = ctx.enter_context(tc.tile_pool(name="const", bufs=1))
        sb2 = ctx.enter_context(tc.tile_pool(name="sb2", bufs=2))
        sb3 = ctx.enter_context(tc.tile_pool(name="sb3", bufs=3))
        # PSUM budget: 8 banks: s2 + t2 + ab2 + r1 + sm1
        ps_s = ctx.enter_context(tc.tile_pool(name="ps_s", bufs=2, space="PSUM"))
        ps_t = ctx.enter_context(tc.tile_pool(name="ps_t", bufs=2, space="PSUM"))
        ps_ab = ctx.enter_context(tc.tile_pool(name="ps_ab", bufs=2, space="PSUM"))
        ps_r = ctx.enter_context(tc.tile_pool(name="ps_r", bufs=1, space="PSUM"))
        ps_sm = ctx.enter_context(tc.tile_pool(name="ps_sm", bufs=1, space="PSUM"))

        # ---- constants (shared across batches) ----
        w4C_sb = const.tile([D, 1], F32R, name="w4C_sb")
        nc.scalar.dma_start(out=w4C_sb, in_=w4C[:, :].bitcast(F32R))
        w4Q_sb = const.tile([D, 1], F32R, name="w4Q_sb")
        nc.scalar.dma_start(out=w4Q_sb, in_=w4Q[:, :].bitcast(F32R))
        wmlu_sb = const.tile([D, 1], F32, name="wmlu_sb")
        nc.scalar.dma_start(out=wmlu_sb, in_=w4mlu.ap().rearrange("a b d -> d (a b)"))
        ident0 = const.tile([D, D], F32, name="ident0")
        make_identity(nc, ident0)
        identR = const.tile([D, D], F32R, name="identR")
        nc.vector.tensor_copy(identR, ident0)
        identB = const.tile([D, D], BF16, name="identB")
        nc.vector.tensor_copy(identB, ident0)
        id16 = const.tile([16, 16], F32, name="id16")
        make_identity(nc, id16)
        ones_f = const.tile([1, D], F32, name="ones_f")
        nc.vector.memset(ones_f, 1.0)
        one1R = const.tile([1, 1], F32R, name="one1R")
        nc.vector.tensor_copy(one1R, ones_f[:, 0:1])
        ones_row = const.tile([1, D], F32R, name="ones_row")
        nc.vector.tensor_copy(ones_row, ones_f)
        ones_col = const.tile([D, 1], F32, name="ones_col")
        nc.vector.memset(ones_col, 1.0)

        for b in range(BL):
            with (tc.high_priority(HIPRI_OFF) if b > 0 else nullcontext()):
                # ---- loads (sync queue: C; scalar: Q; swdge: masks) ----
                Q_sb = sb2.tile([D, LQ], F32R, name="Q_sb")
                nc.scalar.dma_start(out=Q_sb, in_=Qi[b, :, :].bitcast(F32R))
                qm_row = sb2.tile([1, LQ], I32, name="qm_row")
                nc.gpsimd.dma_start(out=qm_row, in_=QMi[b, :].unsqueeze(0))
                cm_pi = sb2.tile([16, 128], I32, name="cm_pi")
                nc.gpsimd.dma_start(out=cm_pi,
                                    in_=CMi[b, :].rearrange("(p i) -> p i", p=16))
                C_sb = sb2.tile([D, LC], F32R, name="C_sb")
                nc.sync.dma_start(out=C_sb[:, 0:1024],
                                  in_=Ci[b, :, 0:1024].bitcast(F32R))
                nc.sync.dma_start(out=C_sb[:, 1024:2048],
                                  in_=Ci[b, :, 1024:2048].bitcast(F32R))
                # Ct output block ships as soon as C is resident
                nc.sync.dma_start(out=out[b, 0:128, :], in_=C_sb.bitcast(F32))

                # ---- mask prep ----
                # qneg_row = NEG*(1-qm) as a [1, LQ] row
                qneg_row = sb2.tile([1, LQ], F32R, name="qneg_row")
                nc.vector.tensor_scalar(qneg_row, qm_row, -NEG, NEG,
                                        op0=ALU.mult, op1=ALU.add)
                # cneg tiles [128, 16] via tiny transpose of contiguous [16, 128]
                cn_t = sb2.tile([16, 128], F32, name="cn_t")
                nc.vector.tensor_scalar(cn_t, cm_pi, -NEG, NEG,
                                        op0=ALU.mult, op1=ALU.add)

                # ---- bias_c [128,16] = C^T w4C + cneg (one PSUM group) ----
                c0_p = ps_sm.tile([128, NCT], F32, name="sm")
                for ci in range(NCT):
                    nc.tensor.matmul(c0_p[:, ci : ci + 1],
                                     C_sb.bitcast(F32)[:, ci * 128 : (ci + 1) * 128],
                                     w4C_sb.bitcast(F32), start=(ci == 0),
                                     stop=False)
                nc.tensor.matmul(c0_p, cn_t, id16, is_transpose=True,
                                 start=False, stop=True)
                bias_c = sb2.tile([128, NCT], F32, name="bias_c")
                nc.vector.tensor_copy(bias_c, c0_p)

                # ---- qterm_row [1, LQ] = q1 + qneg  (PSUM-accumulated) ----
                qt_p = ps_sm.tile([1, LQ], F32, name="sm")
                nc.tensor.matmul(qt_p, w4Q_sb, Q_sb, start=True, stop=False)
                nc.tensor.matmul(qt_p, one1R, qneg_row, start=False, stop=True)
                qterm_row = sb2.tile([1, LQ], F32R, name="qterm_row")
                nc.scalar.activation(qterm_row, qt_p, AF.Copy)

                # ---- Cw = C * w4mlu ----
                Cw = sb2.tile([D, LC], F32R, name="Cw")
                for cj in range(NCJ):
                    sl = slice(cj * 512, (cj + 1) * 512)
                    if cj % 2 == 0:
                        nc.scalar.activation(Cw[:, sl], C_sb[:, sl].bitcast(F32),
                                             AF.Copy, scale=wmlu_sb[:, 0:1])
                    else:
                        nc.vector.tensor_scalar_mul(Cw[:, sl],
                                                    C_sb[:, sl].bitcast(F32),
                                                    wmlu_sb[:, 0:1])

                # ---- Ct tiles (plain, bf16) + ones column ----
                Ct_sb = sb2.tile([128, NCT, CTS], BF16, name="Ct_sb")
                nc.gpsimd.tensor_copy(
                    Ct_sb[:, :, 128:129],
                    ones_col[:, 0:1].unsqueeze(1).to_broadcast((128, NCT, 1)))
                for ci in range(NCT):
                    tp = ps_t.tile([128, 128], F32R, name="pt")
                    nc.tensor.transpose(tp, C_sb[:, ci * 128 : (ci + 1) * 128],
                                        identR)
                    cpe = nc.vector if ci % 2 == 0 else nc.scalar
                    if ci % 2 == 0:
                        nc.vector.tensor_copy(Ct_sb[:, ci, 0:128], tp.bitcast(F32))
                    else:
                        nc.scalar.activation(Ct_sb[:, ci, 0:128], tp.bitcast(F32),
                                             AF.Copy)

                # ---- Qt tiles (plain, bf16 weights for A) ----
                Qt_sb = sb2.tile([128, NQT, 128], BF16, name="Qt_sb")
                for qi in range(NQT):
                    tpq = ps_t.tile([128, 128], F32R, name="pt")
                    nc.tensor.transpose(tpq, Q_sb[:, qi * 128 : (qi + 1) * 128],
                                        identR)
                    nc.vector.tensor_copy(Qt_sb[:, qi, :], tpq.bitcast(F32))

                # ---- S phase: X = exp(S'), rowsum via ActE accum ----
                X = sb2.tile([128, NCT, LQ], BF16, name="X")
                rowsum = sb2.tile([128, NCT], F32, name="rowsum")
                rcp = sb2.tile([128, NCT], F32, name="rcp")
                diags = sb2.tile([128, NCT, 128], BF16, name="diags")
                for ci in range(NCT):
                    sp = ps_s.tile([128, LQ], F32, name="sp")
                    nc.tensor.matmul(sp, ones_row, qterm_row,
                                     start=True, stop=False)
                    nc.tensor.matmul(sp, Cw[:, ci * 128 : (ci + 1) * 128], Q_sb,
                                     start=False, stop=True)
                    nc.scalar.activation(X[:, ci, :], sp, AF.Exp,
                                         bias=bias_c[:, ci : ci + 1], scale=1.0,
                                         accum_out=rowsum[:, ci : ci + 1])
                    if ci % 8 == 7:
                        g = slice(ci - 7, ci + 1)
                        # eps guard: all-masked rows divide 0/eps -> 0, not NaN
                        nc.vector.tensor_scalar_add(rowsum[:, g], rowsum[:, g],
                                                    EPS)
                        nc.vector.reciprocal(rcp[:, g], rowsum[:, g])
                        for ck in range(ci - 7, ci + 1):
                            nc.vector.tensor_scalar_mul(diags[:, ck, :], identB,
                                                        rcp[:, ck : ck + 1])

            # ---- R phase (+ S1t transposes sharing the X stationaries) ----
            R_sb = sb2.tile([128, NQT, 128], BF16, name="R_sb")
            rs2 = sb2.tile([128, NQT], F32, name="rs2")
            S1t = sb2.tile([128, NQT, LC], BF16, name="S1t")
            for qi in range(NQT):
                rp = ps_r.tile([128, CTS], F32, name="rp")
                for cj in range(NCJ):
                    tp = ps_t.tile([128, 512], F32, name="pt")
                    for k in range(4):
                        ci = cj * 4 + k
                        xblk = X[:, ci, qi * 128 : (qi + 1) * 128]
                        nc.tensor.matmul(rp, xblk, Ct_sb[:, ci, 0:CTS],
                                         start=(ci == 0), stop=(ci == NCT - 1))
                        nc.tensor.matmul(tp[:, k * 128 : (k + 1) * 128],
                                         xblk, diags[:, ci, :],
                                         start=True, stop=True)
                    cpeng = (nc.vector, nc.scalar, nc.vector, nc.scalar)[cj]
                    if cpeng is nc.scalar:
                        nc.scalar.activation(S1t[:, qi, cj * 512 : (cj + 1) * 512],
                                             tp, AF.Copy)
                    else:
                        cpeng.tensor_copy(S1t[:, qi, cj * 512 : (cj + 1) * 512], tp)
                nc.vector.tensor_scalar_add(rs2[:, qi : qi + 1], rp[:, 128:129],
                                            EPS)
                nc.vector.reciprocal(rs2[:, qi : qi + 1], rs2[:, qi : qi + 1])
                nc.vector.tensor_scalar_mul(R_sb[:, qi, :], rp[:, 0:128],
                                            rs2[:, qi : qi + 1])

            # ---- A/B matmuls + products + stores ----
            for cj in range(NCJ):
                sl = slice(cj * 512, (cj + 1) * 512)
                pa = ps_ab.tile([128, 512], F32, name="pab")
                pb = ps_ab.tile([128, 512], F32, name="pab")
                for qi in range(NQT):
                    nc.tensor.matmul(pa, Qt_sb[:, qi, :], S1t[:, qi, sl],
                                     start=(qi == 0), stop=(qi == NQT - 1))
                for qi in range(NQT):
                    nc.tensor.matmul(pb, R_sb[:, qi, :], S1t[:, qi, sl],
                                     start=(qi == 0), stop=(qi == NQT - 1))
                cab = sb3.tile([128, 4, 512], F32, name="cab")
                last = b == BL - 1 and cj == NCJ - 1
                nc.scalar.activation(cab[:, 0, :], pa, AF.Copy)
                if last:
                    # tail: skip the B staging copy, read PSUM directly
                    nc.vector.tensor_tensor(cab[:, 1, :],
                                            C_sb[:, sl].bitcast(F32), pa, ALU.mult)
                    nc.vector.tensor_tensor(cab[:, 2, :],
                                            C_sb[:, sl].bitcast(F32), pb, ALU.mult)
                else:
                    nc.scalar.activation(cab[:, 3, :], pb, AF.Copy)
                    nc.gpsimd.tensor_tensor(cab[:, 1, :],
                                            C_sb[:, sl].bitcast(F32),
                                            cab[:, 0, :], ALU.mult)
                    nc.gpsimd.tensor_tensor(cab[:, 2, :],
                                            C_sb[:, sl].bitcast(F32),
                                            cab[:, 3, :], ALU.mult)
                nc.sync.dma_start(
                    out=out[b, 128:512, sl].rearrange("(r p) c -> p r c", p=128),
                    in_=cab[:, 0:3, :])

    nc.finalize()
    return nc


_NC = None


def _get_nc():
    global _NC
    if _NC is None:
        _NC = _build_nc()
    return _NC


def kernel(C, Q, Cmask, Qmask, w4C, w4Q, w4mlu, bias, _trace=False):
    C = np.ascontiguousarray(np.asarray(C, dtype=np.float32))
    Q = np.ascontiguousarray(np.asarray(Q, dtype=np.float32))
    Cmask = np.ascontiguousarray(np.asarray(Cmask, dtype=np.int32))
    Qmask = np.ascontiguousarray(np.asarray(Qmask, dtype=np.int32))
    w4C = np.ascontiguousarray(np.asarray(w4C, dtype=np.float32))
    w4Q = np.ascontiguousarray(np.asarray(w4Q, dtype=np.float32))
    w4mlu = np.ascontiguousarray(np.asarray(w4mlu, dtype=np.float32))
    bias = np.ascontiguousarray(np.asarray(bias, dtype=np.float32))

    nc = _get_nc()
    in_maps = []
    for i in range(NCORES):
        s = slice(i * BL, (i + 1) * BL)
        in_maps.append({
            "C": C[s], "Q": Q[s], "Cmask": Cmask[s], "Qmask": Qmask[s],
            "w4C": w4C, "w4Q": w4Q, "w4mlu": w4mlu, "bias": bias,
        })
    res = run_bass_kernel_spmd(nc, in_maps, core_ids=list(range(NCORES)),
                               trace=_trace)
    out = np.concatenate([r["out"] for r in res.results], axis=0)
    if _trace:
        kernel._last_results = res
    return out
